# revision 1
# baseline (speedup 1.0000x reference)
# Causal multi-head self-attention with RoPE, sharded over 8 NeuronCores.
#
# Sharding: core c handles batch b = c//4 and head group g = c%4 (heads
# 4g..4g+3).  Each core computes its 4 heads' attention output y (256
# features of the 1024-wide y matrix) and the partial output projection
# partial = y @ w_out[:, 256g:256g+256].T; the host sums the 4 partials
# per batch.
#
# Device layouts (prepared on host):
#   xT   [128, 8, 2048]  x[b].T chunked: [p, oc, s] = x[b][s, oc*128+p]
#   wq   [128, 8, 256]   (permuted+scaled q rows).T chunked, te-contiguous
#   wk   [128, 8, 256]   (permuted k rows).T chunked
#   wv   [128, 8, 260]   v rows with a zero pad col per head (ones slot)
#   wo   [128, 2, 1024]  w_out[:, 256g:256g+256].T chunked
#   cos/sin [128, 2048]  cos[s, pair].T tiled 4x vertically
#   mask [128, 1024]     bf16 causal staircase: mask[r, c] = (r <= c-384)
#
# In-kernel dataflow (S.T everywhere; q always on the free axis):
#   QT/KT (rotated) [128, 2, 2048]: chunk0 = even-pair ("te") feats of all
#   4 heads (32 partitions per head), chunk1 = odd ("to") feats.
#   S.T[k,q] tiles from 2 accumulating K=32 matmuls per head,
#   tile_position=(32h, 0) packs the 4 heads into distinct PE row groups.
#   P.T = exp(S.T) in bf16 (no max subtraction: |scores| < ~4 by
#   construction), causal masking by 0/1 multiply after exp.
#   AV: yT_aug[65, 512] += V_aug[k, 65].T @ P.T per k block; row 64 is the
#   softmax denominator (ones column fused into V).
import numpy as np
import ml_dtypes

import concourse.bass as bass
import concourse.mybir as mybir
import concourse.tile as tile
from concourse import bacc
from concourse.bass import ds, ts

B, S, DM, H, DH = 2, 2048, 1024, 16, 64
NCORE, HPC, NSB = 8, 4, 4
F32, BF16, F32R = mybir.dt.float32, mybir.dt.bfloat16, mybir.dt.float32r
EXP = mybir.ActivationFunctionType.Exp
COPYF = mybir.ActivationFunctionType.Copy


def build_nc():
    nc = bacc.Bacc("TRN2", target_bir_lowering=False, debug=False, num_devices=1)
    xT = nc.dram_tensor("xT", [128, 8, S], F32, kind="ExternalInput").ap()
    wq = nc.dram_tensor("wq", [128, 8, 256], F32, kind="ExternalInput").ap()
    wk = nc.dram_tensor("wk", [128, 8, 256], F32, kind="ExternalInput").ap()
    wv = nc.dram_tensor("wv", [128, 8, 260], F32, kind="ExternalInput").ap()
    wo = nc.dram_tensor("wo", [128, 2, 1024], F32, kind="ExternalInput").ap()
    cosd = nc.dram_tensor("cosd", [128, S], F32, kind="ExternalInput").ap()
    sind = nc.dram_tensor("sind", [128, S], F32, kind="ExternalInput").ap()
    maskd = nc.dram_tensor("maskd", [128, 1024], BF16, kind="ExternalInput").ap()
    outd = nc.dram_tensor("out", [S, DM], F32, kind="ExternalOutput").ap()

    with tile.TileContext(nc) as tc:
        with (
            tc.tile_pool(name="persist", bufs=1) as pp,
            tc.tile_pool(name="xpool", bufs=4) as xp,
            tc.tile_pool(name="tmp", bufs=4) as tp,
            tc.tile_pool(name="ptp", bufs=6) as ptp,
            tc.tile_pool(name="misc", bufs=4) as mp,
            tc.tile_pool(name="pbig", bufs=4, space="PSUM") as pbig,
            tc.tile_pool(name="pone", bufs=4, space="PSUM") as pone,
        ):
            wq_s = pp.tile([128, 8, 256], F32)
            wk_s = pp.tile([128, 8, 256], F32)
            wv_s = pp.tile([128, 8, 260], F32)
            wo_s = pp.tile([128, 2, 1024], F32)
            cos_s = pp.tile([128, S], F32)
            sin_s = pp.tile([128, S], F32)
            mask_s = pp.tile([128, 1024], BF16)

            qt = pp.tile([128, 2, S], F32)
            kt = pp.tile([128, 2, S], F32)
            v_s = pp.tile([128, 16, 260], BF16)
            yt = pp.tile([128, 2, S], F32)

            # ---------- Phase A: QKV projection + RoPE ----------
            for sb in range(NSB):
                sbs = ds(sb * 512, 512)
                q_ps = [
                    pbig.tile([128, 512], F32, name=f"qps{c}_{sb}", tag="big")
                    for c in range(2)
                ]
                k_ps = [
                    pbig.tile([128, 512], F32, name=f"kps{c}_{sb}", tag="big")
                    for c in range(2)
                ]
                v_ps = [
                    pone.tile([128, 260], F32, name=f"vps{i}_{sb}", tag="one")
                    for i in range(4)
                ]
                for oc in range(8):
                    xt_t = xp.tile([128, 512], F32, tag="x", name=f"x_{sb}_{oc}")
                    nc.sync.dma_start(xt_t[:], xT[:, oc, sbs])
                    if sb == 0:
                        # interleave weight-chunk loads with the first x tiles
                        # so the first matmul starts ~2us in
                        nc.sync.dma_start(wq_s[:, oc, :], wq[:, oc, :])
                        nc.sync.dma_start(wk_s[:, oc, :], wk[:, oc, :])
                        nc.sync.dma_start(wv_s[:, oc, :], wv[:, oc, :])
                        if oc == 6:
                            nc.sync.dma_start(cos_s[:], cosd[:])
                        if oc == 7:
                            nc.sync.dma_start(sin_s[:], sind[:])
                    xr = xt_t.bitcast(F32R)
                    st, sp = oc == 0, oc == 7
                    for c in range(2):
                        nc.tensor.matmul(
                            q_ps[c][:],
                            lhsT=wq_s[:, oc, ts(c, 128)].bitcast(F32R),
                            rhs=xr[:],
                            start=st,
                            stop=sp,
                        )
                        nc.tensor.matmul(
                            k_ps[c][:],
                            lhsT=wk_s[:, oc, ts(c, 128)].bitcast(F32R),
                            rhs=xr[:],
                            start=st,
                            stop=sp,
                        )
                    for i in range(4):
                        nc.tensor.matmul(
                            v_ps[i][:],
                            lhsT=xt_t[:, ts(i, 128)].bitcast(F32R),
                            rhs=wv_s[:, oc, :].bitcast(F32R),
                            start=st,
                            stop=sp,
                        )
                cosb, sinb = cos_s[:, sbs], sin_s[:, sbs]
                for nm, dst, src in (("q", qt, q_ps), ("k", kt, k_ps)):
                    # rotate in the te-contiguous layout (full-width DVE ops),
                    # then DMA-shuffle into the head-contiguous persistent
                    # layout: dst chunk j holds heads 2j (parts 0-63) and
                    # 2j+1 (parts 64-127), each as [te(32) | to(32)].
                    rot = tp.tile([128, 2, 512], F32, tag="rot", name=f"{nm}rot_{sb}")
                    t1 = tp.tile([128, 512], F32, tag="t1", name=f"{nm}t1_{sb}")
                    t2 = tp.tile([128, 512], F32, tag="t2", name=f"{nm}t2_{sb}")
                    nc.vector.tensor_mul(t1[:], src[0][:], cosb)
                    nc.vector.tensor_mul(t2[:], src[1][:], sinb)
                    nc.vector.tensor_sub(rot[:, 0, :], t1[:], t2[:])
                    t3 = tp.tile([128, 512], F32, tag="t1", name=f"{nm}t3_{sb}")
                    t4 = tp.tile([128, 512], F32, tag="t2", name=f"{nm}t4_{sb}")
                    nc.vector.tensor_mul(t3[:], src[0][:], sinb)
                    nc.vector.tensor_mul(t4[:], src[1][:], cosb)
                    nc.vector.tensor_add(rot[:, 1, :], t3[:], t4[:])
                    for h in range(HPC):
                        # [32, 2, 512] -> [64, 512]: dst partition 2r+t, i.e.
                        # head-contiguous with re/ro interleaved (same order
                        # for q and k, so the dot product is unchanged)
                        nc.gpsimd.dma_start(
                            dst[ds(64 * (h % 2), 64), h // 2, sbs],
                            rot[ds(32 * h, 32), :, :],
                        )
                # V copies on the (idle) scalar engine
                for i in range(4):
                    nc.scalar.activation(v_s[:, sb * 4 + i, :], v_ps[i][:], COPYF)
                if sb == 0:
                    nc.sync.dma_start(mask_s[:], maskd[:])
                    nc.sync.dma_start(wo_s[:], wo[:])
            for h in range(HPC):
                nc.vector.memset(v_s[:, :, h * 65 + 64], 1.0)

            # ---------- Phase B+C: attention and output projection per sb ----
            for sb in range(NSB):
                sbs = ds(sb * 512, 512)
                nkb = (sb + 1) * 4
                y_ps = [
                    pone.tile([65, 512], F32, tag="one", name=f"y{h}_{sb}")
                    for h in range(HPC)
                ]
                prev = None  # (pt tiles, col offset, kb)
                for kb in range(nkb):
                    kbs = ds(kb * 128, 128)
                    # columns before the diagonal are fully masked; trim them
                    # off QK / exp / mask / AV for the diagonal blocks
                    q0 = max(0, (kb - sb * 4) * 128)
                    w = 512 - q0
                    qs = ds(sb * 512 + q0, w)
                    s_ps = []
                    for h in range(HPC):
                        s_t = pbig.tile(
                            [128, 512], F32, tag="big", name=f"s{h}_{sb}_{kb}"
                        )
                        p0, c = 64 * (h % 2), h // 2
                        hp = ds(p0, 64)
                        nc.tensor.matmul(
                            s_t[:, q0:],
                            lhsT=kt[hp, c, kbs].bitcast(F32R),
                            rhs=qt[hp, c, qs].bitcast(F32R),
                            start=True,
                            stop=True,
                            tile_position=(p0, 0),
                        )
                        s_ps.append(s_t)
                    # AV for the previous k block overlaps this block's exp
                    if prev is not None:
                        ppt, pq0, pkb = prev
                        for h in range(HPC):
                            nc.tensor.matmul(
                                y_ps[h][:, pq0:],
                                lhsT=v_s[:, pkb, ds(65 * h, 65)],
                                rhs=ppt[h][:, pq0:],
                                start=(pkb == 0),
                                stop=False,
                            )
                    cur_pt = []
                    for h in range(HPC):
                        pt_t = ptp.tile(
                            [128, 512], BF16, tag="pt", name=f"pt{h}_{sb}_{kb}"
                        )
                        nc.scalar.activation(pt_t[:, q0:], s_ps[h][:, q0:], EXP)
                        if kb >= sb * 4:
                            nc.vector.tensor_mul(
                                pt_t[:, q0:],
                                pt_t[:, q0:],
                                mask_s[:, ds(384, w)],
                            )
                        cur_pt.append(pt_t)
                    prev = (cur_pt, q0, kb)
                ppt, pq0, pkb = prev
                for h in range(HPC):
                    nc.tensor.matmul(
                        y_ps[h][:, pq0:],
                        lhsT=v_s[:, pkb, ds(65 * h, 65)],
                        rhs=ppt[h][:, pq0:],
                        start=(pkb == 0),
                        stop=True,
                    )
                for h in range(HPC):
                    rc = mp.tile([1, 512], F32, tag="rc", name=f"rc{h}_{sb}")
                    nc.vector.reciprocal(rc[:], y_ps[h][64:65, :])
                    rb = mp.tile([64, 512], F32, tag="rb", name=f"rb{h}_{sb}")
                    nc.gpsimd.partition_broadcast(rb[:], rc[:])
                    c, po = h // 2, (h % 2) * 64
                    nc.vector.tensor_mul(
                        yt[ds(po, 64), c, sbs], y_ps[h][0:64, :], rb[:]
                    )
                # output projection for this super-block (4 row blocks)
                for ii in range(4):
                    i = sb * 4 + ii
                    for nh in range(2):
                        o_ps = pbig.tile(
                            [128, 512], F32, tag="big", name=f"o_{i}_{nh}"
                        )
                        for c in range(2):
                            nc.tensor.matmul(
                                o_ps[:],
                                lhsT=yt[:, c, ts(i, 128)].bitcast(F32R),
                                rhs=wo_s[:, c, ds(nh * 512, 512)].bitcast(F32R),
                                start=(c == 0),
                                stop=(c == 1),
                            )
                        o_sb = tp.tile(
                            [128, 512], F32, tag="osb", name=f"osb_{i}_{nh}"
                        )
                        nc.scalar.activation(o_sb[:], o_ps[:], COPYF)
                        nc.sync.dma_start(
                            outd[ts(i, 128), ds(nh * 512, 512)], o_sb[:]
                        )

    nc.compile()
    return nc


def _chunk_T(a):
    # [R, C] -> [128, R//128, C]: out[p, oc, c] = a[oc*128 + p, c]
    r, c = a.shape
    return np.ascontiguousarray(a.reshape(r // 128, 128, c).transpose(1, 0, 2))


def prepare_core_inputs(x, wqkv, w_out, cos, sin, token_positions):
    """Returns list of 8 in_maps (core c: batch c//4, head group c%4)."""
    x = np.asarray(x, np.float32)
    wqkv = np.asarray(wqkv, np.float32)
    w_out = np.asarray(w_out, np.float32)
    cos_pos = np.asarray(cos, np.float32)[np.asarray(token_positions)]
    sin_pos = np.asarray(sin, np.float32)[np.asarray(token_positions)]

    cos_sb = np.ascontiguousarray(np.tile(cos_pos.T, (4, 1)))  # [128, S]
    sin_sb = np.ascontiguousarray(np.tile(sin_pos.T, (4, 1)))
    rr = np.arange(128)[:, None]
    cc = np.arange(1024)[None, :]
    mask_sb = (rr <= cc - 384).astype(ml_dtypes.bfloat16)

    in_maps = []
    for core in range(NCORE):
        b, g = core // 4, core % 4
        xT_sb = _chunk_T(x[b].T)  # [128, 8, 2048]

        perm = []
        for par in range(2):  # te / to
            for l in range(4):
                for rq in range(32):
                    perm.append((4 * g + l) * 64 + 2 * rq + par)
        perm = np.array(perm)
        wq_rows = wqkv[perm] * 0.125  # fold 1/sqrt(DH)
        wk_rows = wqkv[1024 + perm]
        wq_sb = _chunk_T(np.ascontiguousarray(wq_rows.T))  # [128, 8, 256]
        wk_sb = _chunk_T(np.ascontiguousarray(wk_rows.T))

        wv_aug = np.zeros((260, 1024), np.float32)
        for l in range(4):
            r0 = 2048 + (4 * g + l) * 64
            wv_aug[l * 65 : l * 65 + 64] = wqkv[r0 : r0 + 64]
        wv_sb = _chunk_T(np.ascontiguousarray(wv_aug.T))  # [128, 8, 260]

        wo_cols = w_out[:, 256 * g : 256 * g + 256]  # [1024, 256]
        wo_sb = _chunk_T(np.ascontiguousarray(wo_cols.T))  # [128, 2, 1024]

        in_maps.append(
            {
                "xT": xT_sb,
                "wq": wq_sb,
                "wk": wk_sb,
                "wv": wv_sb,
                "wo": wo_sb,
                "cosd": cos_sb,
                "sind": sin_sb,
                "maskd": mask_sb,
            }
        )
    return in_maps


def assemble_output(results):
    out = np.zeros((B, S, DM), np.float32)
    for core in range(NCORE):
        out[core // 4] += results[core]["out"]
    return out


_NC_CACHE = {}


def get_nc():
    if "nc" not in _NC_CACHE:
        _NC_CACHE["nc"] = build_nc()
    return _NC_CACHE["nc"]


def kernel(x, wqkv, w_out, cos, sin, token_positions):
    from concourse import bass_utils

    nc = get_nc()
    in_maps = prepare_core_inputs(x, wqkv, w_out, cos, sin, token_positions)
    res = bass_utils.run_bass_kernel_spmd(nc, in_maps, core_ids=list(range(NCORE)))
    return assemble_output(res.results)
